# revision 1
# baseline (speedup 1.0000x reference)
"""Trainium2 Bass kernel for nn_BatchProgramCC (siamese program classifier).

Network (per side): embed tokens -> per-statement conv (Wc) + tanh + masked
max over tokens -> bidirectional GRU over statements -> residual -> max over
time. Head: softmax(h2l @ |lvec - rvec|).

Distribution: pure data-parallel over the batch (B=32) across 8 NeuronCores
(4 program-pairs per core); weights/embedding replicated. Each core runs an
identical NEFF on its own batch shard; the host concatenates the 8 output
shards. No collectives.

Device-side dataflow per core:
  * embedding gather via the DMA-gather-transpose path (two passes over a
    zero-row-padded bf16 table to cover V=50000 with int16 indices); invalid
    tokens re-gather the statement's first token so the per-statement max is
    unchanged (no -inf masking needed anywhere).
  * Wc projection as bf16 matmuls (E on partitions), per-statement token max
    via grouped reduce_max straight out of PSUM, tanh(+bias) on ACT,
    statement-validity mask multiply.
  * xw = Wih @ enc precomputed for all steps/gates; the GRU runs 128
    sequential steps with fwd+bwd and both sides merged into one 16-lane
    chain ([H on partitions, lanes on free]); per step an identity-matmul
    injects xw into PSUM, 6 small matmuls accumulate Whh h, sigmoid/tanh on
    ACT, gate algebra on DVE.
  * residual + max-pool over time, |l - r|, 2-class softmax via sigmoid.
"""

import os
import numpy as np
import ml_dtypes

# ---------------------------------------------------------------- sizes ----
V, E, D, H, L = 50000, 128, 256, 128, 2
B, S, T = 32, 128, 32
NCORES = 8
PB = B // NCORES            # programs per core = 4
NLANE = 2 * PB              # sequences per direction per core = 8 (side-major)
NSTMT = NLANE * S           # statements per core = 1024
NTOK = NSTMT * T            # tokens per core = 32768
SPLIT = 32766               # ids < SPLIT go to gather pass A
PTAB_ROWS = 2 + V           # 50002 (two zero rows)
NCHUNK = 8                  # gather chunks
CTOK = NTOK // NCHUNK       # tokens per chunk = 4096

BF16 = ml_dtypes.bfloat16

_cache = {}


# ------------------------------------------------------------ device IR ----
def _build_program():
    from contextlib import ExitStack
    import concourse.mybir as mybir
    import concourse.tile as tile
    from concourse import bacc
    from concourse.masks import make_identity

    dt = mybir.dt
    Alu = mybir.AluOpType
    Act = mybir.ActivationFunctionType

    nc = bacc.Bacc("TRN2", target_bir_lowering=False, debug=False,
                   num_devices=NCORES)

    NG = NTOK // 128   # 256 free cols of the gather-layout int tiles

    # ---- DRAM tensors (per-core views; same names on every core) ----
    ptab = nc.dram_tensor("ptab", [PTAB_ROWS, E], dt.bfloat16, kind="ExternalInput")
    toksg = nc.dram_tensor("toksg", [128, NG], dt.int32, kind="ExternalInput")
    tok0g = nc.dram_tensor("tok0g", [128, NG], dt.int32, kind="ExternalInput")
    tleng = nc.dram_tensor("tleng", [128, NG], dt.int32, kind="ExternalInput")
    plens = nc.dram_tensor("plens", [NLANE], dt.int32, kind="ExternalInput")
    wcT = nc.dram_tensor("wcT", [E, 2, 128], dt.bfloat16, kind="ExternalInput")
    wcb = nc.dram_tensor("wcb", [128, 2], dt.float32, kind="ExternalInput")
    wihT = nc.dram_tensor("wihT", [2, 2, 128, 3 * H], dt.bfloat16, kind="ExternalInput")
    whhT = nc.dram_tensor("whhT", [2, H, 3 * H], dt.bfloat16, kind="ExternalInput")
    bih3 = nc.dram_tensor("bih3", [2, 128, 3], dt.float32, kind="ExternalInput")
    bhh3 = nc.dram_tensor("bhh3", [2, 128, 3], dt.float32, kind="ExternalInput")
    h2lT = nc.dram_tensor("h2lT", [2, 128, L], dt.float32, kind="ExternalInput")
    h2lb = nc.dram_tensor("h2lb", [1, L], dt.float32, kind="ExternalInput")
    rcol = nc.dram_tensor("rcol", [128, 1], dt.float32, kind="ExternalInput")
    out_d = nc.dram_tensor("probs", [PB, L], dt.float32, kind="ExternalOutput")

    cut = os.environ.get("BPCC_CUT", "")
    with tile.TileContext(nc) as tc, ExitStack() as ctx:
        persist = ctx.enter_context(tc.tile_pool(name="persist", bufs=1))
        dram = ctx.enter_context(tc.tile_pool(name="dram", bufs=1, space="DRAM"))

        # ---------------- persistent SBUF buffers ----------------
        def ptile(shape, dtype, name):
            return persist.tile(shape, dtype, tag=name, name=name)

        w_wcT = ptile([E, 2, 128], dt.bfloat16, "w_wcT")
        w_wcb = ptile([128, 2], dt.float32, "w_wcb")
        w_wihT = ptile([128, 2, 2, 3 * H], dt.bfloat16, "w_wihT")
        w_whhT = ptile([128, 2, 3 * H], dt.bfloat16, "w_whhT")
        b_ih = ptile([128, 2, 3], dt.float32, "b_ih")
        b_hh = ptile([128, 2, 3], dt.float32, "b_hh")
        b_xw = ptile([128, 2, 3], dt.float32, "b_xw")
        w_h2lT = ptile([128, 2, L], dt.float32, "w_h2lT")
        w_h2lb = ptile([1, L], dt.float32, "w_h2lb")
        pl_sb = ptile([NLANE, 1], dt.int32, "pl_sb")

        tk_sb = ptile([128, NG], dt.int32, "tk_sb")
        t0_sb = ptile([128, NG], dt.int32, "t0_sb")
        tl_sb = ptile([128, NG], dt.int32, "tl_sb")
        idxA = ptile([128, NG], dt.int16, "idxA")
        idxB = ptile([128, NG], dt.int16, "idxB")
        idxAr = ptile([128, NTOK // 16], dt.int16, "idxAr")
        idxBr = ptile([128, NTOK // 16], dt.int16, "idxBr")

        ident = ptile([128, 128], dt.bfloat16, "ident")
        smask = ptile([128, NSTMT], dt.bfloat16, "smask")
        stmt_pre = ptile([128, 2, NSTMT], dt.float32, "stmt_pre")
        encT = ptile([128, 2, NSTMT], dt.bfloat16, "encT")
        # xw blocks per direction: gates [r, z, n] x [step, lane]
        # (backward direction is stored step-reversed)
        xw_all = ptile([128, 2, 3, S, NLANE], dt.bfloat16, "xw_all")
        outbuf = ptile([128, S + 1, 2 * NLANE], dt.bfloat16, "outbuf")
        mx = ptile([128, 2, NLANE], dt.float32, "mx")
        ad = ptile([128, 2, PB], dt.float32, "ad")
        head_sb = ptile([2, PB], dt.float32, "head_sb")
        head_row = ptile([1, 2 * PB], dt.float32, "head_row")
        probs_sb = ptile([1, 2 * PB], dt.float32, "probs_sb")
        rcol_sb = ptile([128, 1], dt.float32, "rcol_sb")

        # ---------------- weight / input loads ----------------
        nc.sync.dma_start(w_wcT[:], wcT[:])
        nc.sync.dma_start(w_wcb[:], wcb[:])
        nc.sync.dma_start(w_wihT[:], wihT[:].rearrange("d k p g -> p d k g"))
        nc.sync.dma_start(w_whhT[:], whhT[:].rearrange("d p g -> p d g"))
        nc.sync.dma_start(b_ih[:], bih3[:].rearrange("d p g -> p d g"))
        nc.sync.dma_start(b_hh[:], bhh3[:].rearrange("d p g -> p d g"))
        nc.sync.dma_start(w_h2lT[:], h2lT[:].rearrange("k p l -> p k l"))
        nc.sync.dma_start(w_h2lb[:], h2lb[:])
        nc.sync.dma_start(pl_sb[:], plens[:].rearrange("(p o) -> p o", o=1))
        nc.sync.dma_start(tk_sb[:], toksg[:])
        nc.sync.dma_start(t0_sb[:], tok0g[:])
        nc.sync.dma_start(tl_sb[:], tleng[:])
        nc.sync.dma_start(rcol_sb[:], rcol[:])

        make_identity(nc, ident[:])

        # combined bias for the xw fold: r/z get bih+bhh, n gets bih only
        nc.vector.tensor_tensor(b_xw[:], b_ih[:], b_hh[:], Alu.add)
        nc.vector.tensor_copy(b_xw[:, :, 2], b_ih[:, :, 2])

        # ---------------- gather index construction (fp32 arithmetic) ----
        # idx layout: partition p = 16j + r (j = token chunk), col f;
        # element (p, f) = token at stream position 4096j + 16f + r.
        scratch = ctx.enter_context(tc.tile_pool(name="scratch", bufs=1))
        t_t = scratch.tile([128, NG], dt.int32, tag="t_t")
        # 16*(f%2); + (p%16) below  ->  token index t within the statement
        nc.gpsimd.iota(t_t[:].rearrange("p (a b) -> p a b", b=2),
                       pattern=[[0, NG // 2], [16, 2]], base=0,
                       channel_multiplier=0)
        t_f = scratch.tile([128, NG], dt.float32, tag="t_f")
        nc.vector.tensor_copy(t_f[:], t_t[:])
        nc.vector.tensor_single_scalar(out=t_f[:], in_=t_f[:],
                                       scalar=rcol_sb[:], op=Alu.add)
        tk_f = scratch.tile([128, NG], dt.float32, tag="tk_f")
        nc.vector.tensor_copy(tk_f[:], tk_sb[:])
        t0_f = scratch.tile([128, NG], dt.float32, tag="t0_f")
        nc.vector.tensor_copy(t0_f[:], t0_sb[:])
        tl_f = scratch.tile([128, NG], dt.float32, tag="tl_f")
        nc.vector.tensor_copy(tl_f[:], tl_sb[:])
        vm = scratch.tile([128, NG], dt.float32, tag="vm")
        nc.vector.tensor_tensor(vm[:], t_f[:], tl_f[:], Alu.is_lt)
        iv = scratch.tile([128, NG], dt.float32, tag="iv")
        nc.vector.tensor_tensor(iv[:], t_f[:], tl_f[:], Alu.is_ge)
        eff = scratch.tile([128, NG], dt.float32, tag="eff")
        nc.vector.tensor_tensor(eff[:], tk_f[:], vm[:], Alu.mult)
        nc.vector.tensor_tensor(iv[:], t0_f[:], iv[:], Alu.mult)
        nc.vector.tensor_tensor(eff[:], eff[:], iv[:], Alu.add)
        mA = scratch.tile([128, NG], dt.float32, tag="mA")
        nc.vector.tensor_single_scalar(out=mA[:], in_=eff[:],
                                       scalar=float(SPLIT), op=Alu.is_lt)
        ia32 = scratch.tile([128, NG], dt.float32, tag="ia32")
        nc.vector.scalar_tensor_tensor(out=ia32[:], in0=eff[:], scalar=1.0,
                                       in1=mA[:], op0=Alu.add, op1=Alu.mult)
        nc.vector.tensor_copy(idxA[:], ia32[:])
        nc.vector.tensor_single_scalar(out=mA[:], in_=eff[:],
                                       scalar=float(SPLIT), op=Alu.is_ge)
        nc.vector.scalar_tensor_tensor(out=ia32[:], in0=eff[:],
                                       scalar=float(SPLIT - 1), in1=mA[:],
                                       op0=Alu.subtract, op1=Alu.mult)
        nc.vector.tensor_copy(idxB[:], ia32[:])

        # replicate the 16-partition index wrap 8x across partition groups
        # (dma_gather consumes [128, n/16] with the data repeated per group)
        idxAd = dram.tile([128, NG], dt.int16, tag="idxAd")
        idxBd = dram.tile([128, NG], dt.int16, tag="idxBd")
        nc.sync.dma_start(idxAd[:], idxA[:])
        nc.sync.dma_start(idxBd[:], idxB[:])
        for (dsrc, drep) in ((idxAd, idxAr), (idxBd, idxBr)):
            src = dsrc[:].rearrange("(j r) f -> r j f", r=16)
            for k in range(8):
                nc.sync.dma_start(
                    drep[16 * k:16 * (k + 1), :].rearrange(
                        "p (j f) -> p j f", f=NG),
                    src)

        # ---------------- statement-validity mask ----------------
        # msp[lane, s] = (plen[lane] >= S - s); bounce through DRAM to
        # replicate across the 128 D-partitions.
        thr = scratch.tile([NLANE, S], dt.int32, tag="thr")
        nc.gpsimd.iota(thr[:], pattern=[[-1, S]], base=S, channel_multiplier=0)
        thr_f = scratch.tile([NLANE, S], dt.float32, tag="thr_f")
        nc.vector.tensor_copy(thr_f[:], thr[:])
        pl_f = scratch.tile([NLANE, 1], dt.float32, tag="pl_f")
        nc.vector.tensor_copy(pl_f[:], pl_sb[:])
        msp_bf = scratch.tile([NLANE, S], dt.bfloat16, tag="msp_bf")
        nc.vector.tensor_single_scalar(out=msp_bf[:], in_=thr_f[:],
                                       scalar=pl_f[:], op=Alu.is_le)
        smask_d = dram.tile([NLANE, S], dt.bfloat16, tag="smask_d")
        nc.sync.dma_start(smask_d[:], msp_bf[:])
        nc.sync.dma_start(
            smask[:],
            smask_d[:].rearrange("l s -> (l s)").unsqueeze(0)
            .broadcast_to([128, NSTMT]))

        # ---------------- embed: gather + Wc + token-max ----------------
        with tc.tile_pool(name="gx", bufs=3) as gx, \
             tc.tile_pool(name="pemb", bufs=6, space="PSUM") as pemb:
            for j in range(NCHUNK):
                xa = gx.tile([128, 1, CTOK], dt.bfloat16, tag="xa")
                xb = gx.tile([128, 1, CTOK], dt.bfloat16, tag="xb")
                nc.gpsimd.dma_gather(
                    out_ap=xa[:], in_ap=ptab[0:SPLIT + 1, :],
                    idxs_ap=idxAr[:, NG * j:NG * (j + 1)],
                    num_idxs=CTOK, num_idxs_reg=CTOK, elem_size=E,
                    transpose=True, single_packet=False)
                nc.gpsimd.dma_gather(
                    out_ap=xb[:], in_ap=ptab[SPLIT + 1:PTAB_ROWS, :],
                    idxs_ap=idxBr[:, NG * j:NG * (j + 1)],
                    num_idxs=CTOK, num_idxs_reg=CTOK, elem_size=E,
                    transpose=True, single_packet=False)
                for dh in range(2):
                    for mt in range(CTOK // 512):
                        ps = pemb.tile([128, 512], dt.float32, tag="pe")
                        sl = slice(512 * mt, 512 * (mt + 1))
                        nc.tensor.matmul(ps[:], w_wcT[:, dh, :],
                                         xa[:, 0, sl], start=True, stop=False)
                        nc.tensor.matmul(ps[:], w_wcT[:, dh, :],
                                         xb[:, 0, sl], start=False, stop=True)
                        c0 = j * (NSTMT // NCHUNK) + mt * 16
                        nc.vector.tensor_reduce(
                            out=stmt_pre[:, dh, c0:c0 + 16],
                            in_=ps[:].rearrange("p (g t) -> p g t", t=T),
                            axis=mybir.AxisListType.X, op=Alu.max)

        # ---------------- enc = smask * tanh(pre + bias) ----------------
        for dh in range(2):
            nc.scalar.activation(encT[:, dh, :], stmt_pre[:, dh, :], Act.Tanh,
                                 bias=w_wcb[:, dh:dh + 1], scale=1.0)
            nc.vector.tensor_tensor(encT[:, dh, :], encT[:, dh, :], smask[:],
                                    Alu.mult)

        # ---------------- xw precompute ----------------
        # encT cols are lane-major: col = lane*S + s
        with tc.tile_pool(name="pxw", bufs=4, space="PSUM") as pxw:
          if cut not in ("embed",):
            for d in range(2):
                xw_dst = xw_all[:, d]
                for g in range(3):
                    for n2 in range(NSTMT // 512):
                        ps = pxw.tile([128, 512], dt.float32, tag="pxw")
                        for kb in range(2):
                            nc.tensor.matmul(
                                ps[:],
                                w_wihT[:, d, kb, g * H:(g + 1) * H],
                                encT[:, kb, 512 * n2:512 * (n2 + 1)],
                                start=(kb == 0), stop=(kb == 1))
                        # psum cols = (lane, s) lane-major; lanes n2*4..n2*4+4
                        dst = xw_dst[:, g, :, 4 * n2:4 * (n2 + 1)]  # [p,S,4]
                        if d == 1:   # backward direction: store s-reversed
                            dst = dst[:, ::-1, :]
                        dst = dst.transpose([0, 2, 1])  # iterate (lane, s)
                        src = ps[:].rearrange("p (l s) -> p l s", s=S)
                        if g in (0, 2):
                            nc.scalar.activation(dst, src, Act.Identity,
                                                 bias=b_xw[:, d, g:g + 1],
                                                 scale=1.0)
                        else:
                            nc.vector.tensor_single_scalar(
                                out=dst, in_=src, scalar=b_xw[:, d, g:g + 1],
                                op=Alu.add)

        # ---------------- GRU: 128 sequential steps ----------------
        nc.vector.memset(outbuf[:, 0, :], 0.0)
        with tc.tile_pool(name="pgru", bufs=3, space="PSUM") as pgru, \
             tc.tile_pool(name="gsb", bufs=3) as gsb:
          if cut not in ("embed", "xw"):
            NL = NLANE
            for t in range(S):
                for d in range(2):
                    hprev = outbuf[:, t, d * NL:(d + 1) * NL]
                    pp = pgru.tile([128, 3 * NL], dt.float32,
                                   tag=f"pp{d}")
                    rz = pp[:, 0:2 * NL]
                    pn = pp[:, 2 * NL:3 * NL]
                    # open/close the rz accumulation group, then the n group
                    nc.tensor.matmul(rz, ident[:], xw_all[:, d, 0:2, t, :],
                                     start=True, stop=False)
                    nc.tensor.matmul(rz[:, 0:NL], w_whhT[:, d, 0:H], hprev,
                                     start=False, stop=False)
                    nc.tensor.matmul(rz[:, NL:2 * NL], w_whhT[:, d, H:2 * H],
                                     hprev, start=False, stop=True)
                    nc.tensor.matmul(pn, w_whhT[:, d, 2 * H:3 * H], hprev,
                                     start=True, stop=True)
                    rzs = gsb.tile([128, 2 * NL], dt.float32, tag=f"rzs{d}")
                    nc.scalar.activation(rzs[:], rz, Act.Sigmoid)
                    y = gsb.tile([128, NL], dt.float32, tag=f"y{d}")
                    # y = (gh_n + bhh_n) * r
                    nc.vector.scalar_tensor_tensor(
                        out=y[:], in0=pn, scalar=b_hh[:, d, 2:3],
                        in1=rzs[:, 0:NL], op0=Alu.add, op1=Alu.mult)
                    nc.vector.tensor_tensor(y[:], y[:], xw_all[:, d, 2, t, :],
                                            Alu.add)
                    nsb = gsb.tile([128, NL], dt.float32, tag=f"nsb{d}")
                    nc.scalar.activation(nsb[:], y[:], Act.Tanh)
                    d1 = gsb.tile([128, NL], dt.float32, tag=f"d1{d}")
                    nc.gpsimd.tensor_tensor(d1[:], hprev, nsb[:], Alu.subtract)
                    nc.gpsimd.tensor_tensor(d1[:], rzs[:, NL:2 * NL], d1[:],
                                            Alu.mult)
                    nc.gpsimd.tensor_tensor(
                        outbuf[:, t + 1, d * NL:(d + 1) * NL], nsb[:], d1[:],
                        Alu.add)

        # ---------------- residual + time max-pool + head ----------------
        with tc.tile_pool(name="tail", bufs=1) as tail, \
             tc.tile_pool(name="phead", bufs=1, space="PSUM") as phead:
          if cut in ("embed", "xw", "gru"):
            nc.sync.dma_start(out_d[:], stmt_pre[:PB, 0, 0:L])
          if cut == "":
            for dh in range(2):
                go = tail.tile([128, S, NLANE], dt.bfloat16, tag=f"go{dh}")
                hslice = outbuf[:, 1:S + 1, dh * NLANE:(dh + 1) * NLANE]
                ebase = encT[:, dh, :].rearrange("p (l s) -> p s l", s=S)
                if dh == 1:
                    ebase = ebase[:, ::-1, :]   # align enc[s] with h_b[tau]
                nc.vector.tensor_tensor(go[:], hslice, ebase, Alu.add)
                nc.vector.tensor_reduce(
                    out=mx[:, dh, :], in_=go[:].transpose([0, 2, 1]),
                    axis=mybir.AxisListType.X, op=Alu.max)
                # |lvec - rvec|  (lanes 0..3 = side1, 4..7 = side2)
                nc.vector.tensor_tensor(ad[:, dh, :], mx[:, dh, 0:PB],
                                        mx[:, dh, PB:NLANE], Alu.subtract)
                nc.scalar.activation(ad[:, dh, :], ad[:, dh, :], Act.Abs)

            pl = phead.tile([2, PB], dt.float32, tag="ph")
            for dh in range(2):
                nc.tensor.matmul(pl[:], w_h2lT[:, dh, :], ad[:, dh, :],
                                 start=(dh == 0), stop=(dh == 1))
            nc.vector.tensor_copy(head_sb[:], pl[:])
            head_d = dram.tile([2, PB], dt.float32, tag="head_d")
            nc.sync.dma_start(head_d[:], head_sb[:])
            nc.sync.dma_start(
                head_row[:],
                head_d[:].rearrange("l p -> (l p)").unsqueeze(0))
            bd = tail.tile([1, 2], dt.float32, tag="bd")
            nc.vector.tensor_tensor(bd[:, 0:1], w_h2lb[:, 0:1],
                                    w_h2lb[:, 1:2], Alu.subtract)
            nc.vector.tensor_tensor(bd[:, 1:2], w_h2lb[:, 1:2],
                                    w_h2lb[:, 0:1], Alu.subtract)
            df = tail.tile([1, PB], dt.float32, tag="df")
            nc.vector.tensor_tensor(df[:], head_row[:, 0:PB],
                                    head_row[:, PB:2 * PB], Alu.subtract)
            # softmax over 2 classes == sigmoid of the logit difference;
            # write interleaved (prog-major) so the output DMA is contiguous
            pview = probs_sb[:].rearrange("o (p l) -> o p l", l=L)
            nc.scalar.activation(pview[:, :, 0], df[:], Act.Sigmoid,
                                 bias=bd[:, 0:1], scale=1.0)
            nc.scalar.activation(pview[:, :, 1], df[:], Act.Sigmoid,
                                 bias=bd[:, 1:2], scale=-1.0)
            nc.sync.dma_start(out_d[:].rearrange("p l -> (p l)").unsqueeze(0),
                              probs_sb[:])

    nc.compile()
    return nc


def _get_program():
    if "nc" not in _cache:
        _cache["nc"] = _build_program()
    return _cache["nc"]


# ------------------------------------------------------------- host side ----
def _prep_shared(inputs):
    emb = np.asarray(inputs["emb"], np.float32)
    pt = np.zeros((PTAB_ROWS, E), dtype=BF16)
    pt[1:SPLIT + 1] = emb[:SPLIT].astype(BF16)
    pt[SPLIT + 2:] = emb[SPLIT:].astype(BF16)

    wcT = np.ascontiguousarray(
        np.asarray(inputs["Wc_w"], np.float32).T).astype(BF16)
    wcT = wcT.reshape(E, 2, 128)
    wcb = np.ascontiguousarray(
        np.asarray(inputs["Wc_b"], np.float32).reshape(2, 128).T)

    wihT = np.stack([np.asarray(inputs[k], np.float32).T
                     for k in ("wih_f", "wih_b")])
    wihT = np.ascontiguousarray(wihT.reshape(2, 2, 128, 3 * H)).astype(BF16)
    whhT = np.stack([np.asarray(inputs[k], np.float32).T
                     for k in ("whh_f", "whh_b")])
    whhT = np.ascontiguousarray(whhT).astype(BF16)          # [2, 128, 384]
    bih3 = np.ascontiguousarray(np.stack(
        [np.asarray(inputs[k], np.float32).reshape(3, 128).T
         for k in ("bih_f", "bih_b")]))
    bhh3 = np.ascontiguousarray(np.stack(
        [np.asarray(inputs[k], np.float32).reshape(3, 128).T
         for k in ("bhh_f", "bhh_b")]))
    h2lT = np.ascontiguousarray(
        np.asarray(inputs["h2l_w"], np.float32).T.reshape(2, 128, L))
    h2lb = np.asarray(inputs["h2l_b"], np.float32).reshape(1, L)
    rcol = (np.arange(128, dtype=np.float32) % 16).reshape(128, 1)
    return dict(ptab=pt, wcT=wcT, wcb=wcb, wihT=wihT, whhT=whhT,
                bih3=bih3, bhh3=bhh3, h2lT=h2lT, h2lb=h2lb, rcol=rcol)


def _gather_layout(flat32):
    """[NTOK] int32 (statement-major, token-inner) -> [128, NTOK//128] tiles.

    Element (16j + r, f) holds stream position 4096j + 16f + r, matching the
    dma_gather index wrap (idx i of chunk j sits at [(i%16), i//16]).
    """
    tg = flat32.reshape(NCHUNK, CTOK // 16, 16)        # [j, f, r]
    return np.ascontiguousarray(
        tg.transpose(0, 2, 1).reshape(128, NTOK // 128))


def _prep_core(c, inputs):
    sl = slice(PB * c, PB * (c + 1))
    tk = np.stack([np.asarray(inputs["tokens1"][sl]),
                   np.asarray(inputs["tokens2"][sl])])
    tk = tk.astype(np.int32).reshape(NSTMT, T)          # (side,prog,s) x t
    tl = np.stack([np.asarray(inputs["token_lens1"][sl]),
                   np.asarray(inputs["token_lens2"][sl])])
    tl = tl.astype(np.int32).reshape(NSTMT)
    pl = np.stack([np.asarray(inputs["prog_lens1"][sl]),
                   np.asarray(inputs["prog_lens2"][sl])])
    pl = pl.astype(np.int32).reshape(NLANE)

    return dict(toksg=_gather_layout(tk.reshape(NTOK)),
                tok0g=_gather_layout(np.repeat(tk[:, 0], T)),
                tleng=_gather_layout(np.repeat(tl, T)),
                plens=np.ascontiguousarray(pl))


def _make_in_maps(inputs):
    shared = _prep_shared(inputs)
    in_maps = []
    for c in range(NCORES):
        m = dict(shared)
        m.update(_prep_core(c, inputs))
        in_maps.append(m)
    return in_maps


def kernel(**inputs):
    from concourse import bass_utils

    nc = _get_program()
    in_maps = _make_in_maps(inputs)
    res = bass_utils.run_bass_kernel_spmd(nc, in_maps,
                                          core_ids=list(range(NCORES)))
    kernel.last_results = res
    out = np.concatenate([res.results[c]["probs"] for c in range(NCORES)],
                         axis=0)
    return np.ascontiguousarray(out.reshape(B, L, 1).astype(np.float32))


kernel.last_results = None



# revision 21
# speedup vs baseline: 1.5559x; 1.5559x over previous
"""Trainium2 Bass kernel for nn_BatchProgramCC (siamese program classifier).

Network (per side): embed tokens -> per-statement conv (Wc) + tanh + masked
max over tokens -> bidirectional GRU over statements -> residual -> max over
time. Head: softmax(h2l @ |lvec - rvec|).

Distribution: pure data-parallel over the batch (B=32) across 8 NeuronCores
(4 program-pairs per core); weights/embedding replicated. Each core runs an
identical NEFF on its own batch shard; the host concatenates the 8 output
shards. No collectives.

Device-side dataflow per core:
  * embedding gather via the DMA-gather-transpose path (two passes over a
    zero-row-padded bf16 table to cover V=50000 with int16 indices); invalid
    tokens re-gather the statement's first token so the per-statement max is
    unchanged (no -inf masking needed anywhere).
  * Wc projection as bf16 matmuls (E on partitions), per-statement token max
    via grouped reduce_max straight out of PSUM, tanh(+bias) on ACT,
    statement-validity mask multiply.
  * xw = Wih @ enc precomputed for all steps/gates; the GRU runs 128
    sequential steps with fwd+bwd and both sides merged into one 16-lane
    chain ([H on partitions, lanes on free]); per step an identity-matmul
    injects xw into PSUM, 6 small matmuls accumulate Whh h, sigmoid/tanh on
    ACT, gate algebra on DVE.
  * residual + max-pool over time, |l - r|, 2-class softmax via sigmoid.
"""

import os
import numpy as np
import ml_dtypes

# ---------------------------------------------------------------- sizes ----
V, E, D, H, L = 50000, 128, 256, 128, 2
B, S, T = 32, 128, 32
NCORES = 8
PB = B // NCORES            # programs per core = 4
NLANE = 2 * PB              # sequences per direction per core = 8 (side-major)
NSTMT = NLANE * S           # statements per core = 1024
NTOK = NSTMT * T            # tokens per core = 32768
W2 = 2 * NLANE              # lanes across both directions = 16
SPLIT = 32766               # ids < SPLIT go to gather pass A
PTAB_ROWS = 2 + V           # 50002 (two zero rows)
NCHUNK = 8                  # gather chunks
CTOK = NTOK // NCHUNK       # tokens per chunk = 4096

BF16 = ml_dtypes.bfloat16

_cache = {}


# ------------------------------------------------------------ device IR ----
def _build_program():
    from contextlib import ExitStack
    import concourse.mybir as mybir
    import concourse.tile as tile
    from concourse import bacc
    from concourse.masks import make_identity

    dt = mybir.dt
    Alu = mybir.AluOpType
    Act = mybir.ActivationFunctionType

    nc = bacc.Bacc("TRN2", target_bir_lowering=False, debug=False,
                   num_devices=NCORES)

    NG = NTOK // 128   # 256 free cols of the gather-layout int tiles

    # ---- DRAM tensors (per-core views; same names on every core) ----
    ptab = nc.dram_tensor("ptab", [PTAB_ROWS, E], dt.bfloat16, kind="ExternalInput")
    toksg = nc.dram_tensor("toksg", [128, NG], dt.int32, kind="ExternalInput")
    tok0g = nc.dram_tensor("tok0g", [128, NG], dt.int32, kind="ExternalInput")
    tleng = nc.dram_tensor("tleng", [128, NG], dt.int32, kind="ExternalInput")
    plens = nc.dram_tensor("plens", [NLANE], dt.int32, kind="ExternalInput")
    wcT = nc.dram_tensor("wcT", [E, 2, 128], dt.bfloat16, kind="ExternalInput")
    wcb = nc.dram_tensor("wcb", [128, 2], dt.float32, kind="ExternalInput")
    wihT = nc.dram_tensor("wihT", [2, 2, 128, 3 * H], dt.bfloat16, kind="ExternalInput")
    whhT = nc.dram_tensor("whhT", [2, H, 3 * H], dt.bfloat16, kind="ExternalInput")
    bih3 = nc.dram_tensor("bih3", [2, 128, 3], dt.float32, kind="ExternalInput")
    bhh3 = nc.dram_tensor("bhh3", [2, 128, 3], dt.float32, kind="ExternalInput")
    h2lT = nc.dram_tensor("h2lT", [2, 128, L], dt.float32, kind="ExternalInput")
    h2lb = nc.dram_tensor("h2lb", [1, L], dt.float32, kind="ExternalInput")
    rcol = nc.dram_tensor("rcol", [128, 1], dt.float32, kind="ExternalInput")
    out_d = nc.dram_tensor("probs", [PB, L], dt.float32, kind="ExternalOutput")

    cut = os.environ.get("BPCC_CUT", "")
    with tile.TileContext(nc) as tc, ExitStack() as ctx:
        persist = ctx.enter_context(tc.tile_pool(name="persist", bufs=1))
        dram = ctx.enter_context(tc.tile_pool(name="dram", bufs=1, space="DRAM"))

        # ---------------- persistent SBUF buffers ----------------
        def ptile(shape, dtype, name):
            return persist.tile(shape, dtype, tag=name, name=name)

        w_wcT = ptile([E, 2, 128], dt.bfloat16, "w_wcT")
        w_wcb = ptile([128, 2], dt.float32, "w_wcb")
        w_wihT = ptile([128, 2, 2, 3 * H], dt.bfloat16, "w_wihT")
        w_whhT = ptile([128, 2, 3 * H], dt.bfloat16, "w_whhT")
        b_ih = ptile([128, 2, 3], dt.float32, "b_ih")
        b_hh = ptile([128, 2, 3], dt.float32, "b_hh")
        b_xw = ptile([128, 2, 3], dt.float32, "b_xw")
        w_h2lT = ptile([128, 2, L], dt.float32, "w_h2lT")
        w_h2lb = ptile([1, L], dt.float32, "w_h2lb")
        pl_sb = ptile([NLANE, 1], dt.int32, "pl_sb")

        tk_sb = ptile([128, NG], dt.int32, "tk_sb")
        t0_sb = ptile([128, NG], dt.int32, "t0_sb")
        tl_sb = ptile([128, NG], dt.int32, "tl_sb")
        idxA = ptile([128, NG], dt.int16, "idxA")
        idxB = ptile([128, NG], dt.int16, "idxB")
        idxAr = ptile([128, NTOK // 16], dt.int16, "idxAr")
        idxBr = ptile([128, NTOK // 16], dt.int16, "idxBr")

        ident = ptile([128, 128], dt.bfloat16, "ident")
        smask = ptile([128, NSTMT], dt.bfloat16, "smask")
        stmt_pre = ptile([128, 2, NSTMT], dt.float32, "stmt_pre")
        encT = ptile([128, 2, NSTMT], dt.bfloat16, "encT")
        # xw per step: 64 cols = [r_f8 r_b8 | z_f8 z_b8 | (bhh_n,xw_n)x16]
        # (backward direction is stored step-reversed; the n-gate block is
        # interleaved (bhh_n, xw_n) pairs per lane for the psum inject; the
        # z block is NEGATED -- z-gate weights/bias are negated host-side so
        # one sigmoid yields [r | 1-z] at once)
        xw48 = ptile([128, S, 8 * NLANE], dt.bfloat16, "xw48")
        # h stored interleaved: [., t, lane, 1] = h; [., t, lane, 0] = n junk
        outbuf = ptile([128, S + 1, 2 * NLANE, 2], dt.bfloat16, "outbuf")
        # [0, r]x16 pairs then [0, zc]x16 pairs (evens stay zero)
        rzpat = ptile([128, 2 * W2, 2], dt.float32, "rzpat")
        bn_z = ptile([128, 2], dt.float32, "bn_z")
        mx = ptile([128, 2, NLANE], dt.float32, "mx")
        ad = ptile([128, 2, PB], dt.float32, "ad")
        head_sb = ptile([2, PB], dt.float32, "head_sb")
        head_row = ptile([1, 2 * PB], dt.float32, "head_row")
        probs_sb = ptile([1, 2 * PB], dt.float32, "probs_sb")
        rcol_sb = ptile([128, 1], dt.float32, "rcol_sb")

        # ---------------- weight / input loads ----------------
        nc.sync.dma_start(w_wcT[:], wcT[:])
        nc.sync.dma_start(w_wcb[:], wcb[:])
        nc.sync.dma_start(w_wihT[:], wihT[:].rearrange("d k p g -> p d k g"))
        nc.sync.dma_start(w_whhT[:], whhT[:].rearrange("d p g -> p d g"))
        nc.sync.dma_start(b_ih[:], bih3[:].rearrange("d p g -> p d g"))
        nc.sync.dma_start(b_hh[:], bhh3[:].rearrange("d p g -> p d g"))
        nc.sync.dma_start(w_h2lT[:], h2lT[:].rearrange("k p l -> p k l"))
        nc.sync.dma_start(w_h2lb[:], h2lb[:])
        nc.sync.dma_start(pl_sb[:], plens[:].rearrange("(p o) -> p o", o=1))
        nc.sync.dma_start(tk_sb[:], toksg[:])
        nc.sync.dma_start(t0_sb[:], tok0g[:])
        nc.sync.dma_start(tl_sb[:], tleng[:])
        nc.sync.dma_start(rcol_sb[:], rcol[:])

        make_identity(nc, ident[:])

        # combined bias for the xw fold: r/z get bih+bhh, n gets bih only
        nc.vector.tensor_tensor(b_xw[:], b_ih[:], b_hh[:], Alu.add)
        nc.vector.tensor_copy(b_xw[:, :, 2], b_ih[:, :, 2])
        # negated z bias (the z preact is stored negated)
        nc.vector.tensor_scalar_mul(bn_z[:], b_xw[:, :, 1], -1.0)

        # ---------------- gather index construction (fp32 arithmetic) ----
        # idx layout: partition p = 16j + r (j = token chunk), col f;
        # element (p, f) = token at stream position 4096j + 16f + r.
        scratch = ctx.enter_context(tc.tile_pool(name="scratch", bufs=1))
        t_t = scratch.tile([128, NG], dt.int32, tag="t_t")
        # 16*(f%2); + (p%16) below  ->  token index t within the statement
        nc.gpsimd.iota(t_t[:].rearrange("p (a b) -> p a b", b=2),
                       pattern=[[0, NG // 2], [16, 2]], base=0,
                       channel_multiplier=0)
        t_f = scratch.tile([128, NG], dt.float32, tag="t_f")
        nc.vector.tensor_copy(t_f[:], t_t[:])
        nc.vector.tensor_single_scalar(out=t_f[:], in_=t_f[:],
                                       scalar=rcol_sb[:], op=Alu.add)
        tk_f = scratch.tile([128, NG], dt.float32, tag="tk_f")
        nc.vector.tensor_copy(tk_f[:], tk_sb[:])
        t0_f = scratch.tile([128, NG], dt.float32, tag="t0_f")
        nc.vector.tensor_copy(t0_f[:], t0_sb[:])
        tl_f = scratch.tile([128, NG], dt.float32, tag="tl_f")
        nc.vector.tensor_copy(tl_f[:], tl_sb[:])
        vm = scratch.tile([128, NG], dt.float32, tag="vm")
        nc.vector.tensor_tensor(vm[:], t_f[:], tl_f[:], Alu.is_lt)
        iv = scratch.tile([128, NG], dt.float32, tag="iv")
        nc.vector.tensor_tensor(iv[:], t_f[:], tl_f[:], Alu.is_ge)
        eff = scratch.tile([128, NG], dt.float32, tag="eff")
        nc.vector.tensor_tensor(eff[:], tk_f[:], vm[:], Alu.mult)
        nc.vector.tensor_tensor(iv[:], t0_f[:], iv[:], Alu.mult)
        nc.vector.tensor_tensor(eff[:], eff[:], iv[:], Alu.add)
        mA = scratch.tile([128, NG], dt.float32, tag="mA")
        nc.vector.tensor_single_scalar(out=mA[:], in_=eff[:],
                                       scalar=float(SPLIT), op=Alu.is_lt)
        ia32 = scratch.tile([128, NG], dt.float32, tag="ia32")
        nc.vector.scalar_tensor_tensor(out=ia32[:], in0=eff[:], scalar=1.0,
                                       in1=mA[:], op0=Alu.add, op1=Alu.mult)
        nc.vector.tensor_copy(idxA[:], ia32[:])
        nc.vector.tensor_single_scalar(out=mA[:], in_=eff[:],
                                       scalar=float(SPLIT), op=Alu.is_ge)
        nc.vector.scalar_tensor_tensor(out=ia32[:], in0=eff[:],
                                       scalar=float(SPLIT - 1), in1=mA[:],
                                       op0=Alu.subtract, op1=Alu.mult)
        nc.vector.tensor_copy(idxB[:], ia32[:])

        # replicate the 16-partition index wrap 8x across partition groups
        # (dma_gather consumes [128, n/16] with the data repeated per group)
        idxAd = dram.tile([128, NG], dt.int16, tag="idxAd")
        idxBd = dram.tile([128, NG], dt.int16, tag="idxBd")
        nc.sync.dma_start(idxAd[:], idxA[:])
        nc.sync.dma_start(idxBd[:], idxB[:])
        for (dsrc, drep) in ((idxAd, idxAr), (idxBd, idxBr)):
            src = dsrc[:].rearrange("(j r) f -> r j f", r=16)
            for k in range(8):
                nc.sync.dma_start(
                    drep[16 * k:16 * (k + 1), :].rearrange(
                        "p (j f) -> p j f", f=NG),
                    src)

        # ---------------- statement-validity mask ----------------
        # msp[lane, s] = (plen[lane] >= S - s); bounce through DRAM to
        # replicate across the 128 D-partitions.
        thr = scratch.tile([NLANE, S], dt.int32, tag="thr")
        nc.gpsimd.iota(thr[:], pattern=[[-1, S]], base=S, channel_multiplier=0)
        thr_f = scratch.tile([NLANE, S], dt.float32, tag="thr_f")
        nc.vector.tensor_copy(thr_f[:], thr[:])
        pl_f = scratch.tile([NLANE, 1], dt.float32, tag="pl_f")
        nc.vector.tensor_copy(pl_f[:], pl_sb[:])
        msp_bf = scratch.tile([NLANE, S], dt.bfloat16, tag="msp_bf")
        nc.vector.tensor_single_scalar(out=msp_bf[:], in_=thr_f[:],
                                       scalar=pl_f[:], op=Alu.is_le)
        smask_d = dram.tile([NLANE, S], dt.bfloat16, tag="smask_d")
        nc.sync.dma_start(smask_d[:], msp_bf[:])
        nc.sync.dma_start(
            smask[:],
            smask_d[:].rearrange("l s -> (l s)").unsqueeze(0)
            .broadcast_to([128, NSTMT]))

        # ---------------- embed: gather + Wc + token-max ----------------
        with tc.tile_pool(name="gx", bufs=3) as gx, \
             tc.tile_pool(name="pemb", bufs=6, space="PSUM") as pemb:
            for j in range(NCHUNK):
                xa = gx.tile([128, 1, CTOK], dt.bfloat16, tag="xa")
                xb = gx.tile([128, 1, CTOK], dt.bfloat16, tag="xb")
                nc.gpsimd.dma_gather(
                    out_ap=xa[:], in_ap=ptab[0:SPLIT + 1, :],
                    idxs_ap=idxAr[:, NG * j:NG * (j + 1)],
                    num_idxs=CTOK, num_idxs_reg=CTOK, elem_size=E,
                    transpose=True, single_packet=False)
                nc.gpsimd.dma_gather(
                    out_ap=xb[:], in_ap=ptab[SPLIT + 1:PTAB_ROWS, :],
                    idxs_ap=idxBr[:, NG * j:NG * (j + 1)],
                    num_idxs=CTOK, num_idxs_reg=CTOK, elem_size=E,
                    transpose=True, single_packet=False)
                if cut == "gather":
                    continue
                for dh in range(2):
                    for mt in range(CTOK // 512):
                        ps = pemb.tile([128, 512], dt.float32, tag="pe")
                        sl = slice(512 * mt, 512 * (mt + 1))
                        nc.tensor.matmul(ps[:], w_wcT[:, dh, :],
                                         xa[:, 0, sl], start=True, stop=False)
                        nc.tensor.matmul(ps[:], w_wcT[:, dh, :],
                                         xb[:, 0, sl], start=False, stop=True)
                        c0 = j * (NSTMT // NCHUNK) + mt * 16
                        nc.vector.tensor_reduce(
                            out=stmt_pre[:, dh, c0:c0 + 16],
                            in_=ps[:].rearrange("p (g t) -> p g t", t=T),
                            axis=mybir.AxisListType.X, op=Alu.max)

        # ---------------- enc = smask * tanh(pre + bias) ----------------
        if cut != "gather":
            for dh in range(2):
                nc.scalar.activation(encT[:, dh, :], stmt_pre[:, dh, :],
                                     Act.Tanh, bias=w_wcb[:, dh:dh + 1],
                                     scale=1.0)
                nc.vector.tensor_tensor(encT[:, dh, :], encT[:, dh, :],
                                        smask[:], Alu.mult)

        # ---------------- xw precompute ----------------
        # encT cols are lane-major: col = lane*S + s
        with tc.tile_pool(name="pxw", bufs=4, space="PSUM") as pxw:
          if cut not in ("embed", "gather"):
            for d in range(2):
                for g in range(3):
                    for n2 in range(NSTMT // 512):
                        ps = pxw.tile([128, 512], dt.float32, tag="pxw")
                        for kb in range(2):
                            nc.tensor.matmul(
                                ps[:],
                                w_wihT[:, d, kb, g * H:(g + 1) * H],
                                encT[:, kb, 512 * n2:512 * (n2 + 1)],
                                start=(kb == 0), stop=(kb == 1))
                        # psum cols = (lane, s) lane-major; lanes n2*4..n2*4+4
                        l0 = d * NLANE + 4 * n2
                        if g < 2:
                            dst = xw48[:, :, g * W2 + l0:g * W2 + l0 + 4]
                        else:
                            # n gate: odd cols of the interleaved pair block
                            dst = xw48[:, :, 2 * W2:4 * W2].rearrange(
                                "p s (l two) -> p s l two", two=2)[
                                :, :, l0:l0 + 4, 1]
                        if d == 1:   # backward direction: store s-reversed
                            dst = dst[:, ::-1, :]
                        dst = dst.transpose([0, 2, 1])  # iterate (lane, s)
                        src = ps[:].rearrange("p (l s) -> p l s", s=S)
                        if g in (0, 2):
                            nc.scalar.activation(dst, src, Act.Identity,
                                                 bias=b_xw[:, d, g:g + 1],
                                                 scale=1.0)
                        else:
                            # z gate stored negated: -(xw_z + b_z)
                            nc.scalar.activation(dst, src, Act.Identity,
                                                 bias=bn_z[:, d:d + 1],
                                                 scale=-1.0)
            # bhh_n broadcast into the even cols of the n-gate block
            for d in range(2):
                pv = xw48[:, :, 2 * W2:4 * W2].rearrange(
                    "p s (l two) -> p s l two", two=2)
                nc.scalar.activation(pv[:, :, d * NLANE:(d + 1) * NLANE, 0],
                                     pv[:, :, d * NLANE:(d + 1) * NLANE, 1],
                                     Act.Identity, bias=b_hh[:, d, 2:3],
                                     scale=0.0)

        # ---------------- GRU: 128 sequential steps ----------------
        # Both directions run in one instruction stream: 16 lanes
        # (fwd 0:8, bwd 8:16).  Per step the psum holds
        #   [r-preact 16 | z-preact 16 | (pn', xw_n) interleaved 32]
        # with pn' = Whh_n h + bhh_n.  One DVE scan computes
        # y = r*pn' + xw_n (pairwise reset via op0=mult with 0), tanh gives
        # n, a second scan computes h' = zc*n + u (u = z*h from Pool,
        # zc = 1-z straight from a scale=-1 sigmoid).
        nc.vector.memset(outbuf[:, 0, :, :], 0.0)
        nc.vector.memset(rzpat[:], 0.0)
        with tc.tile_pool(name="pgru", bufs=3, space="PSUM") as pgru, \
             tc.tile_pool(name="gsb", bufs=3) as gsb:
          if cut not in ("embed", "xw", "gather"):
            NL = NLANE
            for t in range(S):
                hprev = outbuf[:, t, :, 1]               # [128, 16] strided
                hf = outbuf[:, t, 0:NL, 1]
                hb = outbuf[:, t, NL:W2, 1]
                pp = pgru.tile([128, 4 * W2], dt.float32, tag="pp")
                pnv = pp[:, 2 * W2:4 * W2].rearrange(
                    "p (l two) -> p l two", two=2)
                # one inject opens all three regions (r | -z | (bhh,xw_n));
                # each region is closed by its own Whh matmuls
                nc.tensor.matmul(pp[:], ident[:], xw48[:, t, :],
                                 start=True, stop=False)
                nc.tensor.matmul(pp[:, 0:NL], w_whhT[:, 0, 0:H], hf,
                                 start=False, stop=False)
                nc.tensor.matmul(pp[:, NL:W2], w_whhT[:, 1, 0:H], hb,
                                 start=False, stop=False)
                nc.tensor.matmul(pp[:, W2:W2 + NL], w_whhT[:, 0, H:2 * H],
                                 hf, start=False, stop=False)
                nc.tensor.matmul(pp[:, W2 + NL:2 * W2], w_whhT[:, 1, H:2 * H],
                                 hb, start=False, stop=False)
                nc.tensor.matmul(pnv[:, 0:NL, 0], w_whhT[:, 0, 2 * H:3 * H],
                                 hf, start=False, stop=False)
                nc.tensor.matmul(pnv[:, NL:W2, 0], w_whhT[:, 1, 2 * H:3 * H],
                                 hb, start=False, stop=True)
                # one sigmoid: [r | 1-z] -> rzpat odds (z block is negated)
                nc.scalar.activation(rzpat[:, :, 1], pp[:, 0:2 * W2],
                                     Act.Sigmoid)
                # y scan: odd cols get r*pn' + xw_n
                yb = pgru.tile([128, 2 * W2], dt.float32, tag="yb")
                nc.vector.tensor_tensor_scan(
                    yb[:], rzpat[:, 0:W2, :].rearrange("p l two -> p (l two)"),
                    pp[:, 2 * W2:4 * W2], 0.0, op0=Alu.mult, op1=Alu.add)
                ybv = yb[:].rearrange("p (l two) -> p l two", two=2)
                nu1 = gsb.tile([128, W2, 2], dt.float32, tag="nu1")
                nc.scalar.activation(nu1[:, :, 0], ybv[:, :, 1], Act.Tanh)
                # u = z*h = h - zc*h  (Pool, off the critical path)
                us = gsb.tile([128, W2], dt.float32, tag="us")
                nc.gpsimd.tensor_tensor(us[:], rzpat[:, W2:2 * W2, 1], hprev,
                                        Alu.mult)
                nc.gpsimd.tensor_tensor(nu1[:, :, 1], hprev, us[:],
                                        Alu.subtract)
                # blend scan: odd cols get zc*n + u = h'
                nc.vector.tensor_tensor_scan(
                    outbuf[:, t + 1, :, :].rearrange("p l two -> p (l two)"),
                    rzpat[:, W2:2 * W2, :].rearrange("p l two -> p (l two)"),
                    nu1[:].rearrange("p l two -> p (l two)"),
                    0.0, op0=Alu.mult, op1=Alu.add)

        # ---------------- residual + time max-pool + head ----------------
        with tc.tile_pool(name="tail", bufs=1) as tail, \
             tc.tile_pool(name="phead", bufs=1, space="PSUM") as phead:
          if cut in ("embed", "xw", "gru"):
            nc.sync.dma_start(out_d[:], stmt_pre[:PB, 0, 0:L])
          if cut == "":
            for dh in range(2):
                go = tail.tile([128, S, NLANE], dt.bfloat16, tag=f"go{dh}")
                hslice = outbuf[:, 1:S + 1, dh * NLANE:(dh + 1) * NLANE, 1]
                ebase = encT[:, dh, :].rearrange("p (l s) -> p s l", s=S)
                if dh == 1:
                    ebase = ebase[:, ::-1, :]   # align enc[s] with h_b[tau]
                nc.vector.tensor_tensor(go[:], hslice, ebase, Alu.add)
                nc.vector.tensor_reduce(
                    out=mx[:, dh, :], in_=go[:].transpose([0, 2, 1]),
                    axis=mybir.AxisListType.X, op=Alu.max)
                # |lvec - rvec|  (lanes 0..3 = side1, 4..7 = side2)
                nc.vector.tensor_tensor(ad[:, dh, :], mx[:, dh, 0:PB],
                                        mx[:, dh, PB:NLANE], Alu.subtract)
                nc.scalar.activation(ad[:, dh, :], ad[:, dh, :], Act.Abs)

            pl = phead.tile([2, PB], dt.float32, tag="ph")
            for dh in range(2):
                nc.tensor.matmul(pl[:], w_h2lT[:, dh, :], ad[:, dh, :],
                                 start=(dh == 0), stop=(dh == 1))
            nc.vector.tensor_copy(head_sb[:], pl[:])
            head_d = dram.tile([2, PB], dt.float32, tag="head_d")
            nc.sync.dma_start(head_d[:], head_sb[:])
            nc.sync.dma_start(
                head_row[:],
                head_d[:].rearrange("l p -> (l p)").unsqueeze(0))
            bd = tail.tile([1, 2], dt.float32, tag="bd")
            nc.vector.tensor_tensor(bd[:, 0:1], w_h2lb[:, 0:1],
                                    w_h2lb[:, 1:2], Alu.subtract)
            nc.vector.tensor_tensor(bd[:, 1:2], w_h2lb[:, 1:2],
                                    w_h2lb[:, 0:1], Alu.subtract)
            df = tail.tile([1, PB], dt.float32, tag="df")
            nc.vector.tensor_tensor(df[:], head_row[:, 0:PB],
                                    head_row[:, PB:2 * PB], Alu.subtract)
            # softmax over 2 classes == sigmoid of the logit difference;
            # write interleaved (prog-major) so the output DMA is contiguous
            pview = probs_sb[:].rearrange("o (p l) -> o p l", l=L)
            nc.scalar.activation(pview[:, :, 0], df[:], Act.Sigmoid,
                                 bias=bd[:, 0:1], scale=1.0)
            nc.scalar.activation(pview[:, :, 1], df[:], Act.Sigmoid,
                                 bias=bd[:, 1:2], scale=-1.0)
            nc.sync.dma_start(out_d[:].rearrange("p l -> (p l)").unsqueeze(0),
                              probs_sb[:])

    nc.compile()
    return nc


def _get_program():
    if "nc" not in _cache:
        _cache["nc"] = _build_program()
    return _cache["nc"]


# ------------------------------------------------------------- host side ----
def _prep_shared(inputs):
    emb = np.asarray(inputs["emb"], np.float32)
    pt = np.zeros((PTAB_ROWS, E), dtype=BF16)
    pt[1:SPLIT + 1] = emb[:SPLIT].astype(BF16)
    pt[SPLIT + 2:] = emb[SPLIT:].astype(BF16)

    wcT = np.ascontiguousarray(
        np.asarray(inputs["Wc_w"], np.float32).T).astype(BF16)
    wcT = wcT.reshape(E, 2, 128)
    wcb = np.ascontiguousarray(
        np.asarray(inputs["Wc_b"], np.float32).reshape(2, 128).T)

    wihT = np.stack([np.asarray(inputs[k], np.float32).T
                     for k in ("wih_f", "wih_b")])
    wihT = np.ascontiguousarray(wihT.reshape(2, 2, 128, 3 * H)).astype(BF16)
    whhT = np.stack([np.asarray(inputs[k], np.float32).T
                     for k in ("whh_f", "whh_b")])
    whhT[:, :, 128:256] *= -1.0     # z-gate negated (see device comment)
    whhT = np.ascontiguousarray(whhT).astype(BF16)          # [2, 128, 384]
    bih3 = np.ascontiguousarray(np.stack(
        [np.asarray(inputs[k], np.float32).reshape(3, 128).T
         for k in ("bih_f", "bih_b")]))
    bhh3 = np.ascontiguousarray(np.stack(
        [np.asarray(inputs[k], np.float32).reshape(3, 128).T
         for k in ("bhh_f", "bhh_b")]))
    h2lT = np.ascontiguousarray(
        np.asarray(inputs["h2l_w"], np.float32).T.reshape(2, 128, L))
    h2lb = np.asarray(inputs["h2l_b"], np.float32).reshape(1, L)
    rcol = (np.arange(128, dtype=np.float32) % 16).reshape(128, 1)
    return dict(ptab=pt, wcT=wcT, wcb=wcb, wihT=wihT, whhT=whhT,
                bih3=bih3, bhh3=bhh3, h2lT=h2lT, h2lb=h2lb, rcol=rcol)


def _gather_layout(flat32):
    """[NTOK] int32 (statement-major, token-inner) -> [128, NTOK//128] tiles.

    Element (16j + r, f) holds stream position 4096j + 16f + r, matching the
    dma_gather index wrap (idx i of chunk j sits at [(i%16), i//16]).
    """
    tg = flat32.reshape(NCHUNK, CTOK // 16, 16)        # [j, f, r]
    return np.ascontiguousarray(
        tg.transpose(0, 2, 1).reshape(128, NTOK // 128))


def _prep_core(c, inputs):
    sl = slice(PB * c, PB * (c + 1))
    tk = np.stack([np.asarray(inputs["tokens1"][sl]),
                   np.asarray(inputs["tokens2"][sl])])
    tk = tk.astype(np.int32).reshape(NSTMT, T)          # (side,prog,s) x t
    tl = np.stack([np.asarray(inputs["token_lens1"][sl]),
                   np.asarray(inputs["token_lens2"][sl])])
    tl = tl.astype(np.int32).reshape(NSTMT)
    pl = np.stack([np.asarray(inputs["prog_lens1"][sl]),
                   np.asarray(inputs["prog_lens2"][sl])])
    pl = pl.astype(np.int32).reshape(NLANE)

    return dict(toksg=_gather_layout(tk.reshape(NTOK)),
                tok0g=_gather_layout(np.repeat(tk[:, 0], T)),
                tleng=_gather_layout(np.repeat(tl, T)),
                plens=np.ascontiguousarray(pl))


def _make_in_maps(inputs):
    shared = _prep_shared(inputs)
    in_maps = []
    for c in range(NCORES):
        m = dict(shared)
        m.update(_prep_core(c, inputs))
        in_maps.append(m)
    return in_maps


def kernel(**inputs):
    from concourse import bass_utils

    nc = _get_program()
    in_maps = _make_in_maps(inputs)
    res = bass_utils.run_bass_kernel_spmd(nc, in_maps,
                                          core_ids=list(range(NCORES)))
    kernel.last_results = res
    out = np.concatenate([res.results[c]["probs"] for c in range(NCORES)],
                         axis=0)
    return np.ascontiguousarray(out.reshape(B, L, 1).astype(np.float32))


kernel.last_results = None

